# revision 3
# baseline (speedup 1.0000x reference)
"""Causal multi-head self-attention (B=4, T=2048, C=1024, H=16) on 8 TRN2 NeuronCores.

Sharding: core = b*2 + g  (b = batch 0..3, g = head-group 0..1 of 8 heads each).
Data parallel over batch; tensor parallel over heads (column-parallel W_attn,
row-parallel W_proj). Each core returns a partial (T, C) output; the host sums
the two partials per batch (the TP all-reduce happens in the unshard step).

Per-core device kernel (all bf16 matmuls, f32 accumulation):
  1. qT/kT projection:  qk[c', t] = (W_qk^T x^T)  -- heads on partitions
  2. v projection:      v_aug[t, 65h] with an all-ones column per head
                        (gives the softmax denominator for free in step 4)
  3. causal attention in transposed [s, q] layout:
       S^T = (kT block)^T @ qT chunk   (two K=64 matmuls packed in the
                                        128-row PE array per head pair)
       E = exp(S/8)  (ScalarE, psum->sbuf, no max-subtraction needed:
                      |scores/8| < ~7 so exp is safe in f32/bf16)
       causal mask: multiplicative 0/1 mask on diagonal blocks only
       y^T_aug = sum_j v_aug_j^T @ E_j  (PSUM accumulation; row 64 = denom)
       normalize: r = 1/denom; broadcast r across partitions with a K=1
       matmul against ones; y^T = y^T_aug * r
  4. output projection: out_partial = y^T^T @ W_proj[rows of this group]
"""

import numpy as np
import ml_dtypes

B, T, C, H = 4, 2048, 1024, 16
HS = C // H          # 64
NHL = 8              # local heads per core
KT = C // 128        # 8 contraction subtiles
NQC = T // 512       # 4 query chunks
NTB = T // 128       # 16 t-blocks
Bb16 = ml_dtypes.bfloat16

_CACHE = {}


def _build():
    import concourse.bacc as bacc
    import concourse.tile as tile
    import concourse.mybir as mybir

    BF = mybir.dt.bfloat16
    F32 = mybir.dt.float32
    AF = mybir.ActivationFunctionType

    nc = bacc.Bacc("TRN2", target_bir_lowering=False, debug=False, num_devices=8)
    xT = nc.dram_tensor("xT", [C, T], BF, kind="ExternalInput").ap()
    wqk = nc.dram_tensor("wqk", [C, 1024], BF, kind="ExternalInput").ap()
    wv = nc.dram_tensor("wv", [C, 520], BF, kind="ExternalInput").ap()
    wp = nc.dram_tensor("wp", [512, C], BF, kind="ExternalInput").ap()
    mask = nc.dram_tensor("mask", [128, 2048], BF, kind="ExternalInput").ap()
    out = nc.dram_tensor("out", [T, C], F32, kind="ExternalOutput").ap()

    with tile.TileContext(nc) as tc:
        with tc.tile_pool(name="persist", bufs=1) as persist, \
             tc.tile_pool(name="mm", bufs=2, space="PSUM") as mmpool, \
             tc.tile_pool(name="s", bufs=2, space="PSUM") as spool, \
             tc.tile_pool(name="av", bufs=1, space="PSUM") as avpool, \
             tc.tile_pool(name="bc", bufs=1, space="PSUM") as bcpool, \
             tc.tile_pool(name="e", bufs=3) as epool, \
             tc.tile_pool(name="nrm", bufs=2) as nrmpool, \
             tc.tile_pool(name="osb", bufs=3) as outpool:

            xT_sb = persist.tile([128, KT, T], BF, tag="xT")
            wqk_sb = persist.tile([128, KT, 1024], BF, tag="wqk")
            wv_sb = persist.tile([128, KT, 520], BF, tag="wv")
            wp_sb = persist.tile([128, 4, 1024], BF, tag="wp")
            mask_sb = persist.tile([128, 2048], BF, tag="mask")
            qk_sb = persist.tile([128, 8, T], BF, tag="qk")
            v_sb = persist.tile([128, NTB, 520], BF, tag="v")
            yT_sb = persist.tile([128, 4, T], BF, tag="yT")
            ones_sb = persist.tile([1, 64], BF, tag="ones")

            nc.vector.memset(ones_sb[:], 1.0)
            for k in range(KT):
                nc.sync.dma_start(xT_sb[:, k, :], xT[k * 128:(k + 1) * 128, :])
                nc.sync.dma_start(wqk_sb[:, k, :], wqk[k * 128:(k + 1) * 128, :])
                nc.sync.dma_start(wv_sb[:, k, :], wv[k * 128:(k + 1) * 128, :])
            for k in range(4):
                nc.sync.dma_start(wp_sb[:, k, :], wp[k * 128:(k + 1) * 128, :])
            nc.sync.dma_start(mask_sb[:], mask[:])

            for qc in range(NQC):
                q0 = qc * 512
                # ---- qT/kT projection for this T chunk (col slots 0..3 = q, 4..7 = k)
                for m in range(8):
                    mm_ps = mmpool.tile([128, 512], F32, tag="mm")
                    for k in range(KT):
                        nc.tensor.matmul(
                            mm_ps[:], wqk_sb[:, k, m * 128:(m + 1) * 128],
                            xT_sb[:, k, q0:q0 + 512],
                            start=(k == 0), stop=(k == KT - 1))
                    nc.vector.tensor_copy(qk_sb[:, m, q0:q0 + 512], mm_ps[:])
                # ---- v_aug tiles for this T chunk
                for j in range(4 * qc, 4 * qc + 4):
                    vps = mmpool.tile([128, 512], F32, tag="mm")
                    for k in range(KT):
                        nc.tensor.matmul(
                            vps[:], xT_sb[:, k, j * 128:(j + 1) * 128],
                            wv_sb[:, k, 0:512],
                            start=(k == 0), stop=(k == KT - 1))
                    nc.vector.tensor_copy(v_sb[:, j, 0:512], vps[:])
                    vps2 = mmpool.tile([128, 8], F32, tag="mm")
                    for k in range(KT):
                        nc.tensor.matmul(
                            vps2[:], xT_sb[:, k, j * 128:(j + 1) * 128],
                            wv_sb[:, k, 512:520],
                            start=(k == 0), stop=(k == KT - 1))
                    nc.vector.tensor_copy(v_sb[:, j, 512:520], vps2[:])
                    vones = v_sb[:, j, :].rearrange("p (h e) -> p h e", e=65)[:, :, 64]
                    nc.vector.memset(vones, 1.0)
                # ---- attention for all 8 local heads on this q chunk
                for h in range(NHL):
                    pb = (h % 2) * 64
                    slot = h // 2
                    av_ps = avpool.tile([65, 512], F32, tag="av")
                    njg = 2 * qc + 2
                    for jg in range(njg):
                        s_ps = spool.tile([128, 1024], F32, tag="s")
                        for jj in range(2):
                            j = jg * 2 + jj
                            nc.tensor.matmul(
                                s_ps[:, jj * 512:(jj + 1) * 512],
                                qk_sb[pb:pb + 64, 4 + slot, j * 128:(j + 1) * 128],
                                qk_sb[pb:pb + 64, slot, q0:q0 + 512],
                                start=True, stop=True)
                        e_sb = epool.tile([128, 1024], BF, tag="e")
                        nc.scalar.activation(e_sb[:], s_ps[:], AF.Exp, scale=0.125)
                        if jg == njg - 2:
                            nc.vector.tensor_mul(e_sb[:], e_sb[:], mask_sb[:, 0:1024])
                        elif jg == njg - 1:
                            nc.vector.tensor_mul(e_sb[:], e_sb[:], mask_sb[:, 1024:2048])
                        for jj in range(2):
                            j = jg * 2 + jj
                            nc.tensor.matmul(
                                av_ps[:], v_sb[:, j, h * 65:h * 65 + 65],
                                e_sb[:, jj * 512:(jj + 1) * 512],
                                start=(j == 0), stop=(j == 4 * qc + 3))
                    # normalize by the accumulated denominator (row 64)
                    r_sb = nrmpool.tile([1, 512], BF, tag="r")
                    with nc.allow_low_precision(reason="softmax denom bf16"):
                        nc.vector.reciprocal(r_sb[:], av_ps[64:65, :])
                    bc_ps = bcpool.tile([64, 512], F32, tag="bc")
                    nc.tensor.matmul(bc_ps[:], ones_sb[:], r_sb[:], start=True, stop=True)
                    bc_sb = nrmpool.tile([64, 512], BF, tag="bcs")
                    nc.scalar.copy(bc_sb[:], bc_ps[:])
                    with nc.allow_low_precision(reason="attention y bf16"):
                        nc.vector.tensor_mul(
                            yT_sb[pb:pb + 64, slot, q0:q0 + 512],
                            av_ps[0:64, :], bc_sb[:])
                # ---- output projection for this q chunk
                for tt in range(4):
                    t0 = (qc * 4 + tt) * 128
                    for n in range(2):
                        ops = mmpool.tile([128, 512], F32, tag="mm")
                        for cp in range(4):
                            nc.tensor.matmul(
                                ops[:], yT_sb[:, cp, t0:t0 + 128],
                                wp_sb[:, cp, n * 512:(n + 1) * 512],
                                start=(cp == 0), stop=(cp == 3))
                        osb = outpool.tile([128, 512], F32, tag="osb")
                        nc.scalar.copy(osb[:], ops[:])
                        nc.sync.dma_start(out[t0:t0 + 128, n * 512:(n + 1) * 512], osb[:])
    nc.compile()
    return nc


def _get_nc():
    if "nc" not in _CACHE:
        _CACHE["nc"] = _build()
    return _CACHE["nc"]


def _host_prep(x, W_attn, W_proj):
    """Shard + lay out per-core inputs. Returns list of 8 in_maps."""
    x = np.asarray(x, dtype=np.float32)
    W_attn = np.asarray(W_attn, dtype=np.float32)
    W_proj = np.asarray(W_proj, dtype=np.float32)

    # causal mask tiles: mask[s, m*512 + q'] = 1.0 if s <= q' - m*128 else 0
    s_idx = np.arange(128)[:, None]
    q_idx = np.arange(512)[None, :]
    mask = np.concatenate(
        [(s_idx <= q_idx - m * 128).astype(np.float32) for m in range(4)], axis=1
    ).astype(Bb16)

    xT_b = [np.ascontiguousarray(x[b].T).astype(Bb16) for b in range(B)]
    in_maps = []
    for core in range(8):
        b, g = core // 2, core % 2
        c0 = g * 512
        wqk_g = np.concatenate(
            [W_attn[:, c0:c0 + 512], W_attn[:, C + c0:C + c0 + 512]], axis=1
        ).astype(Bb16)
        vbase = W_attn[:, 2 * C + c0:2 * C + c0 + 512]
        wv_g = np.zeros((C, 520), dtype=np.float32)
        for h in range(NHL):
            wv_g[:, h * 65:h * 65 + 64] = vbase[:, h * 64:(h + 1) * 64]
        wp_g = np.ascontiguousarray(W_proj[c0:c0 + 512, :]).astype(Bb16)
        in_maps.append({
            "xT": xT_b[b],
            "wqk": np.ascontiguousarray(wqk_g),
            "wv": wv_g.astype(Bb16),
            "wp": wp_g,
            "mask": mask,
        })
    return in_maps


def kernel(x, W_attn, W_proj):
    from concourse import bass_utils

    nc = _get_nc()
    in_maps = _host_prep(x, W_attn, W_proj)
    res = bass_utils.run_bass_kernel_spmd(nc, in_maps, core_ids=list(range(8)))
    outs = [res.results[c]["out"] for c in range(8)]
    full = np.empty((B, T, C), dtype=np.float32)
    for b in range(B):
        full[b] = outs[2 * b] + outs[2 * b + 1]
    return full


# revision 11
# speedup vs baseline: 1.2263x; 1.2263x over previous
"""Causal multi-head self-attention (B=4, T=2048, C=1024, H=16) on 8 TRN2 NeuronCores.

Sharding: core = b*2 + g  (b = batch 0..3, g = head-group 0..1 of 8 heads each).
Data parallel over batch; tensor parallel over heads (column-parallel W_attn,
row-parallel W_proj). Each core returns a partial (T, C) output; the host sums
the two partials per batch (the TP all-reduce happens in the unshard step).

Per-core device kernel (all bf16 matmuls, f32 accumulation):
  1. qT/kT projection:  qk[c', t] = (W_qk^T x^T)  -- heads on partitions
  2. v projection:      v_aug[t, 65h] with an all-ones column per head
                        (gives the softmax denominator for free in step 4)
  3. causal attention in transposed [s, q] layout:
       S^T = (kT block)^T @ qT chunk   (two K=64 matmuls packed in the
                                        128-row PE array per head pair)
       E = exp(S/8)  (ScalarE, psum->sbuf, no max-subtraction needed:
                      |scores/8| < ~7 so exp is safe in f32/bf16)
       causal mask: multiplicative 0/1 mask on diagonal blocks only
       y^T_aug = sum_j v_aug_j^T @ E_j  (PSUM accumulation; row 64 = denom)
       normalize: r = 1/denom; broadcast r across partitions with a K=1
       matmul against ones; y^T = y^T_aug * r
  4. output projection: out_partial = y^T^T @ W_proj[rows of this group]
"""

import numpy as np
import ml_dtypes

B, T, C, H = 4, 2048, 1024, 16
HS = C // H          # 64
NHL = 8              # local heads per core
KT = C // 128        # 8 contraction subtiles
NQC = T // 512       # 4 query chunks
NTB = T // 128       # 16 t-blocks
Bb16 = ml_dtypes.bfloat16

_CACHE = {}


def _build():
    import concourse.bacc as bacc
    import concourse.tile as tile
    import concourse.mybir as mybir

    BF = mybir.dt.bfloat16
    F32 = mybir.dt.float32
    AF = mybir.ActivationFunctionType

    nc = bacc.Bacc("TRN2", target_bir_lowering=False, debug=False, num_devices=8)
    xT = nc.dram_tensor("xT", [C, T], BF, kind="ExternalInput").ap()
    wqk = nc.dram_tensor("wqk", [C, 1024], BF, kind="ExternalInput").ap()
    wv = nc.dram_tensor("wv", [C, 520], BF, kind="ExternalInput").ap()
    wp = nc.dram_tensor("wp", [512, C], BF, kind="ExternalInput").ap()
    mask = nc.dram_tensor("mask", [128, 2048], BF, kind="ExternalInput").ap()
    out = nc.dram_tensor("out", [T, C], F32, kind="ExternalOutput").ap()

    with tile.TileContext(nc) as tc:
        with tc.tile_pool(name="persist", bufs=1) as persist, \
             tc.tile_pool(name="mm", bufs=2, space="PSUM") as mmpool, \
             tc.tile_pool(name="s", bufs=2, space="PSUM") as spool, \
             tc.tile_pool(name="av", bufs=1, space="PSUM") as avpool, \
             tc.tile_pool(name="bc", bufs=1, space="PSUM") as bcpool, \
             tc.tile_pool(name="e", bufs=3) as epool, \
             tc.tile_pool(name="nrm", bufs=2) as nrmpool, \
             tc.tile_pool(name="osb", bufs=3) as outpool:

            xT_sb = persist.tile([128, KT, T], BF, tag="xT")
            wqk_sb = persist.tile([128, KT, 1024], BF, tag="wqk")
            wv_sb = persist.tile([128, KT, 520], BF, tag="wv")
            wp_sb = persist.tile([128, 4, 1024], BF, tag="wp")
            mask_sb = persist.tile([128, 2048], BF, tag="mask")
            qk_sb = persist.tile([128, 8, T], BF, tag="qk")
            v_sb = persist.tile([128, NTB, 520], BF, tag="v")
            yT_sb = persist.tile([128, 4, T], BF, tag="yT")
            ones_sb = persist.tile([1, 64], BF, tag="ones")

            nc.vector.memset(ones_sb[:], 1.0)
            for k in range(KT):
                nc.sync.dma_start(xT_sb[:, k, :], xT[k * 128:(k + 1) * 128, :])
                nc.sync.dma_start(wqk_sb[:, k, :], wqk[k * 128:(k + 1) * 128, :])
                nc.sync.dma_start(wv_sb[:, k, :], wv[k * 128:(k + 1) * 128, :])
            for k in range(4):
                nc.sync.dma_start(wp_sb[:, k, :], wp[k * 128:(k + 1) * 128, :])
            nc.sync.dma_start(mask_sb[:], mask[:])

            for qc in range(NQC):
                q0 = qc * 512
                # ---- qT/kT projection for this T chunk (col slots 0..3 = q, 4..7 = k)
                for m in range(8):
                    mm_ps = mmpool.tile([128, 512], F32, tag="mm")
                    for k in range(KT):
                        nc.tensor.matmul(
                            mm_ps[:], wqk_sb[:, k, m * 128:(m + 1) * 128],
                            xT_sb[:, k, q0:q0 + 512],
                            start=(k == 0), stop=(k == KT - 1))
                    nc.vector.tensor_copy(qk_sb[:, m, q0:q0 + 512], mm_ps[:])
                # ---- v_aug tiles for this T chunk
                for j in range(4 * qc, 4 * qc + 4):
                    vps = mmpool.tile([128, 512], F32, tag="mm")
                    for k in range(KT):
                        nc.tensor.matmul(
                            vps[:], xT_sb[:, k, j * 128:(j + 1) * 128],
                            wv_sb[:, k, 0:512],
                            start=(k == 0), stop=(k == KT - 1))
                    nc.vector.tensor_copy(v_sb[:, j, 0:512], vps[:])
                    vps2 = mmpool.tile([128, 8], F32, tag="mm")
                    for k in range(KT):
                        nc.tensor.matmul(
                            vps2[:], xT_sb[:, k, j * 128:(j + 1) * 128],
                            wv_sb[:, k, 512:520],
                            start=(k == 0), stop=(k == KT - 1))
                    nc.vector.tensor_copy(v_sb[:, j, 512:520], vps2[:])
                    vones = v_sb[:, j, :].rearrange("p (h e) -> p h e", e=65)[:, :, 64]
                    nc.vector.memset(vones, 1.0)
                # ---- attention for all 8 local heads on this q chunk
                yraw_sb = nrmpool.tile([64, NHL, 512], BF, tag="yraw")
                den8_sb = nrmpool.tile([128, 2, 512], F32, tag="den8")
                nc.vector.memset(den8_sb[:], 1.0)
                for h in range(NHL):
                    av_ps = avpool.tile([65, 512], F32, tag="av")
                    njg = 2 * qc + 2
                    for jg in range(njg):
                        s_ps = spool.tile([128, 1024], F32, tag="s")
                        for jj in range(2):
                            j = jg * 2 + jj
                            nc.tensor.matmul(
                                s_ps[:, jj * 512:(jj + 1) * 512],
                                qk_sb[(h % 2) * 64:(h % 2) * 64 + 64, 4 + h // 2,
                                      j * 128:(j + 1) * 128],
                                qk_sb[(h % 2) * 64:(h % 2) * 64 + 64, h // 2,
                                      q0:q0 + 512],
                                start=True, stop=True)
                        e_sb = epool.tile([128, 1024], BF, tag="e")
                        nc.scalar.activation(e_sb[:], s_ps[:], AF.Exp, scale=0.125)
                        if jg == njg - 2:
                            nc.vector.tensor_mul(e_sb[:], e_sb[:], mask_sb[:, 0:1024])
                        elif jg == njg - 1:
                            nc.vector.tensor_mul(e_sb[:], e_sb[:], mask_sb[:, 1024:2048])
                        for jj in range(2):
                            j = jg * 2 + jj
                            nc.tensor.matmul(
                                av_ps[:], v_sb[:, j, h * 65:h * 65 + 65],
                                e_sb[:, jj * 512:(jj + 1) * 512],
                                start=(j == 0), stop=(j == 4 * qc + 3))
                    # free the av accumulator right away: stash y and denom
                    with nc.allow_low_precision(reason="attention y bf16"):
                        nc.vector.tensor_copy(yraw_sb[:, h, :], av_ps[0:64, :])
                    p32 = (h % 4) * 32
                    nc.vector.tensor_copy(
                        den8_sb[p32:p32 + 1, h // 4, :], av_ps[64:65, :])
                # batched softmax denominators: one approx reciprocal, all 8 heads
                r8_sb = nrmpool.tile([128, 2, 512], F32, tag="r8")
                nc.vector.reciprocal_approx_fast(r8_sb[:], den8_sb[:])
                for h in range(NHL):
                    pb = (h % 2) * 64
                    slot = h // 2
                    p32 = (h % 4) * 32
                    r1_sb = nrmpool.tile([1, 512], BF, tag="r1")
                    with nc.allow_low_precision(reason="softmax denom bf16"):
                        nc.vector.tensor_copy(r1_sb[:], r8_sb[p32:p32 + 1, h // 4, :])
                    bc_ps = bcpool.tile([64, 512], F32, tag="bc")
                    nc.tensor.matmul(bc_ps[:], ones_sb[:], r1_sb[:], start=True, stop=True)
                    with nc.allow_low_precision(reason="attention y bf16"):
                        nc.vector.tensor_mul(
                            yT_sb[pb:pb + 64, slot, q0:q0 + 512],
                            yraw_sb[:, h, :], bc_ps[:])
                # ---- output projection for this q chunk
                for tt in range(4):
                    t0 = (qc * 4 + tt) * 128
                    for n in range(2):
                        ops = mmpool.tile([128, 512], F32, tag="mm")
                        for cp in range(4):
                            nc.tensor.matmul(
                                ops[:], yT_sb[:, cp, t0:t0 + 128],
                                wp_sb[:, cp, n * 512:(n + 1) * 512],
                                start=(cp == 0), stop=(cp == 3))
                        osb = outpool.tile([128, 512], F32, tag="osb")
                        nc.vector.tensor_copy(osb[:], ops[:])
                        nc.sync.dma_start(out[t0:t0 + 128, n * 512:(n + 1) * 512], osb[:])
    nc.compile()
    return nc


def _get_nc():
    if "nc" not in _CACHE:
        _CACHE["nc"] = _build()
    return _CACHE["nc"]


def _host_prep(x, W_attn, W_proj):
    """Shard + lay out per-core inputs. Returns list of 8 in_maps."""
    x = np.asarray(x, dtype=np.float32)
    W_attn = np.asarray(W_attn, dtype=np.float32)
    W_proj = np.asarray(W_proj, dtype=np.float32)

    # causal mask tiles: mask[s, m*512 + q'] = 1.0 if s <= q' - m*128 else 0
    s_idx = np.arange(128)[:, None]
    q_idx = np.arange(512)[None, :]
    mask = np.concatenate(
        [(s_idx <= q_idx - m * 128).astype(np.float32) for m in range(4)], axis=1
    ).astype(Bb16)

    xT_b = [np.ascontiguousarray(x[b].T).astype(Bb16) for b in range(B)]
    in_maps = []
    for core in range(8):
        b, g = core // 2, core % 2
        c0 = g * 512
        wqk_g = np.concatenate(
            [W_attn[:, c0:c0 + 512], W_attn[:, C + c0:C + c0 + 512]], axis=1
        ).astype(Bb16)
        vbase = W_attn[:, 2 * C + c0:2 * C + c0 + 512]
        wv_g = np.zeros((C, 520), dtype=np.float32)
        for h in range(NHL):
            wv_g[:, h * 65:h * 65 + 64] = vbase[:, h * 64:(h + 1) * 64]
        wp_g = np.ascontiguousarray(W_proj[c0:c0 + 512, :]).astype(Bb16)
        in_maps.append({
            "xT": xT_b[b],
            "wqk": np.ascontiguousarray(wqk_g),
            "wv": wv_g.astype(Bb16),
            "wp": wp_g,
            "mask": mask,
        })
    return in_maps


def kernel(x, W_attn, W_proj):
    from concourse import bass_utils

    nc = _get_nc()
    in_maps = _host_prep(x, W_attn, W_proj)
    res = bass_utils.run_bass_kernel_spmd(nc, in_maps, core_ids=list(range(8)))
    outs = [res.results[c]["out"] for c in range(8)]
    full = np.empty((B, T, C), dtype=np.float32)
    for b in range(B):
        full[b] = outs[2 * b] + outs[2 * b + 1]
    return full
